# revision 1
# baseline (speedup 1.0000x reference)
"""Trainium2 Bass kernel for nn_Crop (per-row random crop of audio).

Reference semantics:
    out[i, j] = audio[i, j]             for j <  starts[i]
    out[i, j] = audio[i, j + CROP_NUM]  for j >= starts[i]

Strategy (pure data parallel, 16 rows per core across 8 cores):
Each output row is two contiguous copies of the source row with a
data-dependent split point.  The output row is viewed as 116 blocks of
W=2048 f32 elements.  The host stages each core's audio twice in one
DRAM input: part A = the raw rows (identity blocks), part B = each row
shifted by CROP (so B block k of row i holds audio[i, k*W+CROP : ...]).
One indirect DMA gather per row with host-computed per-lane block
indices then fetches every output block exactly once — lane k reads
its A block when k*W is left of starts[i], else its B block.  All 116
lanes are always valid (measured: OOB-dropped lanes still consume read
bandwidth, so index-validity tricks are not free).  The single block
straddling starts[i] is corrected element-exactly once per core (two
16-lane gathers + predicated copy -> small fix_out tensor, spliced on
the host).  HBM traffic/row = read OUT_LEN + write OUT_LEN = roofline.
"""

import numpy as np

import concourse.bacc as bacc
import concourse.bass as bass
import concourse.mybir as mybir
from concourse import bass_utils
from concourse.bass import IndirectOffsetOnAxis
from concourse.tile import TileContext

# Problem constants (hardcoded per harness contract).
B = 128
L = 262144
CROP = 26214
OUT_LEN = L - CROP  # 235930
N_CORES = 8
R = B // N_CORES  # 16 rows per core

W = 2048                      # block width; L == 128 * W
NP = 128                      # A-part blocks per row
N_BLK = OUT_LEN // W + 1      # 116 blocks cover one output row
N_FULL = OUT_LEN // W         # 115 full blocks
TAIL = OUT_LEN - N_FULL * W   # 410
A_BLKS = (R + 1) * NP         # 2176 A-part blocks (incl. one pad row)
TOT_BLKS = A_BLKS + R * N_BLK  # + 1856 B-part blocks = 4032
BOUNDS = TOT_BLKS - 1
PADW = N_BLK * W              # out row padded to 237568 so every row
                              # store is one 8KB-aligned [116, W] DMA

_programs = {}


def _build_program(reps: int = 1):
    """Build the single SPMD Bass/Tile program (shared by all 8 cores).

    reps > 1 wraps the body in an on-device For_i loop for benchmarking
    (isolates device time from the ~80ms axon dispatch overhead).
    """
    if reps in _programs:
        return _programs[reps]
    nc = bacc.Bacc("TRN2", target_bir_lowering=False, debug=False)

    audio2 = nc.dram_tensor(
        "audio2", [TOT_BLKS, W], mybir.dt.float32, kind="ExternalInput"
    ).ap()
    blk_idx = nc.dram_tensor(
        "blk_idx", [N_BLK, R], mybir.dt.int32, kind="ExternalInput"
    ).ap()
    fix_idx = nc.dram_tensor(
        "fix_idx", [R, 2], mybir.dt.int32, kind="ExternalInput"
    ).ap()
    r_col = nc.dram_tensor(
        "r_col", [R, 1], mybir.dt.float32, kind="ExternalInput"
    ).ap()
    pos_w = nc.dram_tensor(
        "pos_w", [R, W], mybir.dt.float32, kind="ExternalInput"
    ).ap()
    out = nc.dram_tensor(
        "out", [R, PADW], mybir.dt.float32, kind="ExternalOutput"
    ).ap()
    fix_out = nc.dram_tensor(
        "fix_out", [R, W], mybir.dt.float32, kind="ExternalOutput"
    ).ap()

    with TileContext(nc) as tc:
        with (
            tc.tile_pool(name="consts", bufs=1) as consts,
            tc.tile_pool(name="work", bufs=5) as work,
        ):
            blk_idx_sb = consts.tile([N_BLK, R], mybir.dt.int32)
            fix_idx_sb = consts.tile([R, 2], mybir.dt.int32)
            r_col_sb = consts.tile([R, 1], mybir.dt.float32)
            pos_w_sb = consts.tile([R, W], mybir.dt.float32)
            nc.sync.dma_start(out=blk_idx_sb[:], in_=blk_idx[:])
            nc.sync.dma_start(out=fix_idx_sb[:], in_=fix_idx[:])
            nc.sync.dma_start(out=r_col_sb[:], in_=r_col[:])
            nc.sync.dma_start(out=pos_w_sb[:], in_=pos_w[:])

            def body():
                # Boundary fix, once for all R rows: gather each row's
                # identity and shifted boundary block, select elementwise
                # on (local j) < r_i, write to fix_out (host splices it).
                b_id = work.tile([R, W], mybir.dt.float32, tag="b_id")
                b_sh = work.tile([R, W], mybir.dt.float32, tag="b_sh")
                mask16 = work.tile([R, W], mybir.dt.uint8, tag="mask16")
                nc.gpsimd.indirect_dma_start(
                    out=b_id[:],
                    out_offset=None,
                    in_=audio2[:],
                    in_offset=IndirectOffsetOnAxis(ap=fix_idx_sb[:, 0:1], axis=0),
                    element_offset=0,
                    bounds_check=BOUNDS,
                    oob_is_err=False,
                )
                nc.gpsimd.indirect_dma_start(
                    out=b_sh[:],
                    out_offset=None,
                    in_=audio2[:],
                    in_offset=IndirectOffsetOnAxis(ap=fix_idx_sb[:, 1:2], axis=0),
                    element_offset=0,
                    bounds_check=BOUNDS,
                    oob_is_err=False,
                )
                nc.vector.tensor_scalar(
                    mask16[:], pos_w_sb[:], r_col_sb[:, 0:1], None,
                    mybir.AluOpType.is_lt,
                )
                nc.vector.copy_predicated(b_sh[:], mask16[:], b_id[:])
                nc.sync.dma_start(out=fix_out[:], in_=b_sh[:])

                # Main path: one 116-lane gather + two stores per row.
                for i in range(R):
                    t = work.tile([N_BLK, W], mybir.dt.float32, tag="t")
                    nc.gpsimd.indirect_dma_start(
                        out=t[:],
                        out_offset=None,
                        in_=audio2[:],
                        in_offset=IndirectOffsetOnAxis(
                            ap=blk_idx_sb[:, i : i + 1], axis=0
                        ),
                        element_offset=0,
                        bounds_check=BOUNDS,
                        oob_is_err=False,
                    )
                    dst = out[i, :].rearrange("(p w) -> p w", w=W)
                    nc.sync.dma_start(out=dst, in_=t[:])

            if reps == 1:
                body()
            else:
                with tc.For_i(0, reps, 1):
                    body()

    nc.compile()
    _programs[reps] = nc
    return nc


def _host_inputs(audio: np.ndarray, starts: np.ndarray):
    """Shard + build per-core staged audio and index metadata."""
    audio = np.ascontiguousarray(audio, dtype=np.float32)
    starts = np.asarray(starts, dtype=np.int32)

    lane = np.arange(N_BLK, dtype=np.int32)  # [116]
    pos_w = np.broadcast_to(
        np.arange(W, dtype=np.float32)[None, :], (R, W)
    ).copy()

    in_maps = []
    metas = []
    for c in range(N_CORES):
        rows = slice(c * R, (c + 1) * R)
        a_flat = np.concatenate(
            [audio[rows].reshape(-1), np.zeros(L, dtype=np.float32)]
        )  # [(R+1)*L]
        part_a = a_flat[: A_BLKS * W].reshape(A_BLKS, W)
        part_b = np.stack(
            [
                a_flat[i * L + CROP : i * L + CROP + N_BLK * W]
                for i in range(R)
            ]
        ).reshape(R * N_BLK, W)
        audio2 = np.ascontiguousarray(
            np.concatenate([part_a, part_b], axis=0)
        )

        s = starts[rows]  # [R]
        p_star = s // W  # boundary lane per row
        r = (s - p_star * W).astype(np.float32)  # local split in that lane

        row_i = np.arange(R, dtype=np.int32)
        id_base = row_i[None, :] * NP + lane[:, None]          # A-part block
        sh_base = A_BLKS + row_i[None, :] * N_BLK + lane[:, None]  # B-part
        blk_idx = np.where(lane[:, None] < p_star[None, :], id_base, sh_base)

        fix_idx = np.stack(
            [row_i * NP + p_star, A_BLKS + row_i * N_BLK + p_star], axis=1
        )  # [R, 2]

        in_maps.append(
            {
                "audio2": audio2,
                "blk_idx": np.ascontiguousarray(blk_idx.astype(np.int32)),
                "fix_idx": np.ascontiguousarray(fix_idx.astype(np.int32)),
                "r_col": r[:, None].copy(),
                "pos_w": pos_w,
            }
        )
        metas.append(p_star)
    return in_maps, metas


def _unshard(results, metas):
    out = np.empty((B, OUT_LEN), dtype=np.float32)
    for c in range(N_CORES):
        rows = np.array(results[c]["out"][:, :OUT_LEN])
        fix = results[c]["fix_out"]
        p_star = metas[c]
        for i in range(R):
            q = int(p_star[i]) * W
            wv = min(W, OUT_LEN - q)
            rows[i, q : q + wv] = fix[i, :wv]
        out[c * R : (c + 1) * R] = rows
    return out


def kernel(audio: np.ndarray, starts: np.ndarray) -> np.ndarray:
    nc = _build_program()
    in_maps, metas = _host_inputs(audio, starts)
    res = bass_utils.run_bass_kernel_spmd(
        nc, in_maps, core_ids=list(range(N_CORES))
    )
    kernel.last_results = res
    return _unshard(res.results, metas)



# revision 2
# speedup vs baseline: 235.9461x; 235.9461x over previous
"""Trainium2 Bass kernel for nn_Crop (per-row random crop of audio).

Reference semantics:
    out[i, j] = audio[i, j]             for j <  starts[i]
    out[i, j] = audio[i, j + CROP_NUM]  for j >= starts[i]

Strategy (pure data parallel, 16 rows per core across 8 cores):
Each output row is two contiguous copies of the source row with a
data-dependent split point.  The output row is viewed as 116 blocks of
W=2048 f32 elements.  The host stages each core's audio twice in one
DRAM input: part A = the raw rows (identity blocks), part B = each row
shifted by CROP (so B block k of row i holds audio[i, k*W+CROP : ...]).
One indirect DMA gather per row with host-computed per-lane block
indices then fetches every output block exactly once — lane k reads
its A block when k*W is left of starts[i], else its B block.  All 116
lanes are always valid (measured: OOB-dropped lanes still consume read
bandwidth, so index-validity tricks are not free).  The single block
straddling starts[i] is corrected element-exactly (two 16-lane gathers
+ predicated copy) and written into a 117th block column of the SAME
output tensor; the host splices it into place.  A single ExternalOutput
is load-bearing: each extra output tensor costs ~85ms of axon-relay
dispatch overhead per call (measured), dwarfing the ~0.1ms device time.
HBM traffic/row = read OUT_LEN + write OUT_LEN = roofline.
"""

import numpy as np

import concourse.bacc as bacc
import concourse.bass as bass
import concourse.mybir as mybir
from concourse import bass_utils
from concourse.bass import IndirectOffsetOnAxis
from concourse.tile import TileContext

# Problem constants (hardcoded per harness contract).
B = 128
L = 262144
CROP = 26214
OUT_LEN = L - CROP  # 235930
N_CORES = 8
R = B // N_CORES  # 16 rows per core

W = 2048                      # block width; L == 128 * W
NP = 128                      # A-part blocks per row
N_BLK = OUT_LEN // W + 1      # 116 blocks cover one output row
N_FULL = OUT_LEN // W         # 115 full blocks
TAIL = OUT_LEN - N_FULL * W   # 410
A_BLKS = (R + 1) * NP         # 2176 A-part blocks (incl. one pad row)
TOT_BLKS = A_BLKS + R * N_BLK  # + 1856 B-part blocks = 4032
BOUNDS = TOT_BLKS - 1
PADW = (N_BLK + 1) * W        # out row = 116 gather blocks + 1 fix block

_programs = {}


def _build_program(reps: int = 1):
    """Build the single SPMD Bass/Tile program (shared by all 8 cores).

    reps > 1 wraps the body in an on-device For_i loop for benchmarking
    (isolates device time from the axon dispatch overhead).
    """
    if reps in _programs:
        return _programs[reps]
    nc = bacc.Bacc("TRN2", target_bir_lowering=False, debug=False)

    audio2 = nc.dram_tensor(
        "audio2", [TOT_BLKS, W], mybir.dt.float32, kind="ExternalInput"
    ).ap()
    blk_idx = nc.dram_tensor(
        "blk_idx", [N_BLK, R], mybir.dt.int32, kind="ExternalInput"
    ).ap()
    fix_idx = nc.dram_tensor(
        "fix_idx", [R, 2], mybir.dt.int32, kind="ExternalInput"
    ).ap()
    r_col = nc.dram_tensor(
        "r_col", [R, 1], mybir.dt.float32, kind="ExternalInput"
    ).ap()
    pos_w = nc.dram_tensor(
        "pos_w", [R, W], mybir.dt.float32, kind="ExternalInput"
    ).ap()
    out = nc.dram_tensor(
        "out", [R, PADW], mybir.dt.float32, kind="ExternalOutput"
    ).ap()

    with TileContext(nc) as tc:
        with (
            tc.tile_pool(name="consts", bufs=1) as consts,
            tc.tile_pool(name="work", bufs=5) as work,
        ):
            blk_idx_sb = consts.tile([N_BLK, R], mybir.dt.int32)
            fix_idx_sb = consts.tile([R, 2], mybir.dt.int32)
            r_col_sb = consts.tile([R, 1], mybir.dt.float32)
            pos_w_sb = consts.tile([R, W], mybir.dt.float32)
            nc.sync.dma_start(out=blk_idx_sb[:], in_=blk_idx[:])
            nc.sync.dma_start(out=fix_idx_sb[:], in_=fix_idx[:])
            nc.sync.dma_start(out=r_col_sb[:], in_=r_col[:])
            nc.sync.dma_start(out=pos_w_sb[:], in_=pos_w[:])

            def body():
                # Boundary fix, once for all R rows: gather each row's
                # identity and shifted boundary block, select elementwise
                # on (local j) < r_i, write to the 117th block column of
                # out (host splices it into place).
                b_id = work.tile([R, W], mybir.dt.float32, tag="b_id")
                b_sh = work.tile([R, W], mybir.dt.float32, tag="b_sh")
                mask16 = work.tile([R, W], mybir.dt.uint8, tag="mask16")
                nc.gpsimd.indirect_dma_start(
                    out=b_id[:],
                    out_offset=None,
                    in_=audio2[:],
                    in_offset=IndirectOffsetOnAxis(ap=fix_idx_sb[:, 0:1], axis=0),
                    element_offset=0,
                    bounds_check=BOUNDS,
                    oob_is_err=False,
                )
                nc.gpsimd.indirect_dma_start(
                    out=b_sh[:],
                    out_offset=None,
                    in_=audio2[:],
                    in_offset=IndirectOffsetOnAxis(ap=fix_idx_sb[:, 1:2], axis=0),
                    element_offset=0,
                    bounds_check=BOUNDS,
                    oob_is_err=False,
                )
                nc.vector.tensor_scalar(
                    mask16[:], pos_w_sb[:], r_col_sb[:, 0:1], None,
                    mybir.AluOpType.is_lt,
                )
                nc.vector.copy_predicated(b_sh[:], mask16[:], b_id[:])
                nc.sync.dma_start(out=out[:, N_BLK * W :], in_=b_sh[:])

                # Main path: one 116-lane gather + one store per row.
                for i in range(R):
                    t = work.tile([N_BLK, W], mybir.dt.float32, tag="t")
                    nc.gpsimd.indirect_dma_start(
                        out=t[:],
                        out_offset=None,
                        in_=audio2[:],
                        in_offset=IndirectOffsetOnAxis(
                            ap=blk_idx_sb[:, i : i + 1], axis=0
                        ),
                        element_offset=0,
                        bounds_check=BOUNDS,
                        oob_is_err=False,
                    )
                    dst = out[i, : N_BLK * W].rearrange("(p w) -> p w", w=W)
                    nc.sync.dma_start(out=dst, in_=t[:])

            if reps == 1:
                body()
            else:
                with tc.For_i(0, reps, 1):
                    body()

    nc.compile()
    _programs[reps] = nc
    return nc


def _host_inputs(audio: np.ndarray, starts: np.ndarray):
    """Shard + build per-core staged audio and index metadata."""
    audio = np.ascontiguousarray(audio, dtype=np.float32)
    starts = np.asarray(starts, dtype=np.int32)

    lane = np.arange(N_BLK, dtype=np.int32)  # [116]
    pos_w = np.broadcast_to(
        np.arange(W, dtype=np.float32)[None, :], (R, W)
    ).copy()

    in_maps = []
    metas = []
    for c in range(N_CORES):
        rows = slice(c * R, (c + 1) * R)
        a_flat = np.concatenate(
            [audio[rows].reshape(-1), np.zeros(L, dtype=np.float32)]
        )  # [(R+1)*L]
        part_a = a_flat[: A_BLKS * W].reshape(A_BLKS, W)
        part_b = np.stack(
            [
                a_flat[i * L + CROP : i * L + CROP + N_BLK * W]
                for i in range(R)
            ]
        ).reshape(R * N_BLK, W)
        audio2 = np.ascontiguousarray(
            np.concatenate([part_a, part_b], axis=0)
        )

        s = starts[rows]  # [R]
        p_star = s // W  # boundary lane per row
        r = (s - p_star * W).astype(np.float32)  # local split in that lane

        row_i = np.arange(R, dtype=np.int32)
        id_base = row_i[None, :] * NP + lane[:, None]          # A-part block
        sh_base = A_BLKS + row_i[None, :] * N_BLK + lane[:, None]  # B-part
        blk_idx = np.where(lane[:, None] < p_star[None, :], id_base, sh_base)

        fix_idx = np.stack(
            [row_i * NP + p_star, A_BLKS + row_i * N_BLK + p_star], axis=1
        )  # [R, 2]

        in_maps.append(
            {
                "audio2": audio2,
                "blk_idx": np.ascontiguousarray(blk_idx.astype(np.int32)),
                "fix_idx": np.ascontiguousarray(fix_idx.astype(np.int32)),
                "r_col": r[:, None].copy(),
                "pos_w": pos_w,
            }
        )
        metas.append(p_star)
    return in_maps, metas


def _unshard(results, metas):
    out = np.empty((B, OUT_LEN), dtype=np.float32)
    for c in range(N_CORES):
        full = results[c]["out"]
        rows = np.array(full[:, :OUT_LEN])
        fix = full[:, N_BLK * W :]
        p_star = metas[c]
        for i in range(R):
            q = int(p_star[i]) * W
            wv = min(W, OUT_LEN - q)
            rows[i, q : q + wv] = fix[i, :wv]
        out[c * R : (c + 1) * R] = rows
    return out


def kernel(audio: np.ndarray, starts: np.ndarray) -> np.ndarray:
    nc = _build_program()
    in_maps, metas = _host_inputs(audio, starts)
    res = bass_utils.run_bass_kernel_spmd(
        nc, in_maps, core_ids=list(range(N_CORES))
    )
    kernel.last_results = res
    return _unshard(res.results, metas)


# revision 6
# speedup vs baseline: 236.3597x; 1.0018x over previous
"""Trainium2 Bass kernel for nn_Crop (per-row random crop of audio).

Reference semantics:
    out[i, j] = audio[i, j]             for j <  starts[i]
    out[i, j] = audio[i, j + CROP_NUM]  for j >= starts[i]

Strategy (pure data parallel, 16 rows per core across 8 cores):
out[i] is an elementwise select between the row read at offset 0
(identity) and the row read at offset CROP (shifted), keyed on
global position < starts[i].  Per row: two plain strided DMA loads
into [116, 2048] SBUF tiles (no indirection — the shifted view is just
audio[i, CROP : CROP + 116*2048] reshaped), one tensor_scalar is_lt
building the mask from a precomputed global-position iota against the
row's start (per-partition scalar), one copy_predicated blending the
identity values over the shifted tile, one store.  Handles every lane
including the straddling block exactly — no boundary fixup, no host
splice.  The last row's shifted load is split to stay in bounds; other
rows over-read into the next row (harmless, host-trimmed).

A single ExternalOutput tensor is load-bearing: each extra output costs
~85ms of axon-relay dispatch overhead per call (measured), dwarfing the
~0.3ms device time.  Device HBM traffic/row = read ~2x OUT_LEN + write
OUT_LEN; at ~200GB/s per-core DMA this is ~0.25ms, invisible under the
~41ms dispatch floor.  Inputs are zero-copy views of the caller's audio.
"""

import numpy as np

import concourse.bacc as bacc
import concourse.bass as bass
import concourse.mybir as mybir
from concourse import bass_utils
from concourse.tile import TileContext

# Problem constants (hardcoded per harness contract).
B = 128
L = 262144
CROP = 26214
OUT_LEN = L - CROP  # 235930
N_CORES = 8
R = B // N_CORES  # 16 rows per core

W = 2048                      # block width; L == 128 * W
N_BLK = OUT_LEN // W + 1      # 116 blocks cover one output row
N_FULL = OUT_LEN // W         # 115 full blocks
TAIL = OUT_LEN - N_FULL * W   # 410
PADW = N_BLK * W              # out row padded to 237568 so every row
                              # store is one 8KB-aligned [116, W] DMA

_programs = {}


def _build_program(reps: int = 1):
    """Build the single SPMD Bass/Tile program (shared by all 8 cores).

    reps > 1 wraps the body in an on-device For_i loop for benchmarking
    (isolates device time from the axon dispatch overhead).
    """
    if reps in _programs:
        return _programs[reps]
    nc = bacc.Bacc("TRN2", target_bir_lowering=False, debug=False)

    audio = nc.dram_tensor(
        "audio", [R * L], mybir.dt.float32, kind="ExternalInput"
    ).ap()
    s_rep = nc.dram_tensor(
        "s_rep", [N_BLK, R], mybir.dt.float32, kind="ExternalInput"
    ).ap()
    glob_pos = nc.dram_tensor(
        "glob_pos", [N_BLK, W], mybir.dt.float32, kind="ExternalInput"
    ).ap()
    out = nc.dram_tensor(
        "out", [R, PADW], mybir.dt.float32, kind="ExternalOutput"
    ).ap()

    with TileContext(nc) as tc:
        with (
            tc.tile_pool(name="consts", bufs=1) as consts,
            tc.tile_pool(name="work", bufs=4) as work,
        ):
            s_rep_sb = consts.tile([N_BLK, R], mybir.dt.float32)
            glob_pos_sb = consts.tile([N_BLK, W], mybir.dt.float32)
            nc.sync.dma_start(out=s_rep_sb[:], in_=s_rep[:])
            nc.sync.dma_start(out=glob_pos_sb[:], in_=glob_pos[:])

            def body():
                for i in range(R):
                    t_sh = work.tile([N_BLK, W], mybir.dt.float32, tag="sh")
                    t_id = work.tile([N_BLK, W], mybir.dt.float32, tag="id")
                    mask = work.tile([N_BLK, W], mybir.dt.uint8, tag="mask")
                    # shifted view: audio[i*L + CROP + k*W + j]
                    base = i * L + CROP
                    if i < R - 1:
                        # over-reads 1638 elems into row i+1 (lane 115,
                        # cols >= 410): harmless, host-trimmed
                        src = audio[base : base + N_BLK * W].rearrange(
                            "(p w) -> p w", w=W
                        )
                        nc.sync.dma_start(out=t_sh[:], in_=src)
                    else:
                        # last row: stay inside the input tensor
                        src = audio[base : base + N_FULL * W].rearrange(
                            "(p w) -> p w", w=W
                        )
                        nc.sync.dma_start(out=t_sh[:N_FULL, :], in_=src)
                        tail = audio[
                            base + N_FULL * W : base + N_FULL * W + TAIL
                        ].rearrange("(p w) -> p w", w=TAIL)
                        nc.sync.dma_start(
                            out=t_sh[N_FULL : N_FULL + 1, :TAIL], in_=tail
                        )
                        # lane 115 cols >= 410 are never selected by a
                        # valid mask, but keep them defined
                        nc.sync.dma_start(
                            out=t_sh[N_FULL : N_FULL + 1, TAIL:],
                            in_=audio[i * L : i * L + W - TAIL].rearrange(
                                "(p w) -> p w", w=W - TAIL
                            ),
                        )
                    # identity view: audio[i*L + k*W + j]
                    src_id = audio[i * L : i * L + N_BLK * W].rearrange(
                        "(p w) -> p w", w=W
                    )
                    nc.sync.dma_start(out=t_id[:], in_=src_id)
                    # mask = (k*W + j) < starts[i]  -> take identity there
                    nc.vector.tensor_scalar(
                        mask[:], glob_pos_sb[:], s_rep_sb[:, i : i + 1],
                        None, mybir.AluOpType.is_lt,
                    )
                    nc.vector.copy_predicated(t_sh[:], mask[:], t_id[:])
                    dst = out[i, :].rearrange("(p w) -> p w", w=W)
                    nc.sync.dma_start(out=dst, in_=t_sh[:])

            if reps == 1:
                body()
            else:
                with tc.For_i(0, reps, 1):
                    body()

    nc.compile()
    _programs[reps] = nc
    return nc


_GLOB_POS = None


def _host_inputs(audio: np.ndarray, starts: np.ndarray):
    """Shard per core: audio slices are zero-copy views; consts are tiny."""
    global _GLOB_POS
    audio = np.ascontiguousarray(audio, dtype=np.float32)
    starts = np.asarray(starts, dtype=np.int32)

    if _GLOB_POS is None:
        _GLOB_POS = (
            np.arange(N_BLK, dtype=np.float32)[:, None] * W
            + np.arange(W, dtype=np.float32)[None, :]
        )  # [116, 2048], exact in f32 (max 237567 < 2^24)

    in_maps = []
    for c in range(N_CORES):
        rows = slice(c * R, (c + 1) * R)
        s_rep = np.broadcast_to(
            starts[rows].astype(np.float32)[None, :], (N_BLK, R)
        ).copy()
        in_maps.append(
            {
                "audio": audio[rows].reshape(-1),  # zero-copy view
                "s_rep": s_rep,
                "glob_pos": _GLOB_POS,
            }
        )
    return in_maps


def _unshard(results):
    out = np.empty((B, OUT_LEN), dtype=np.float32)
    for c in range(N_CORES):
        out[c * R : (c + 1) * R] = results[c]["out"][:, :OUT_LEN]
    return out


def kernel(audio: np.ndarray, starts: np.ndarray) -> np.ndarray:
    nc = _build_program()
    in_maps = _host_inputs(audio, starts)
    res = bass_utils.run_bass_kernel_spmd(
        nc, in_maps, core_ids=list(range(N_CORES))
    )
    kernel.last_results = res
    return _unshard(res.results)


# revision 7
# speedup vs baseline: 242.4577x; 1.0258x over previous
"""Trainium2 Bass kernel for nn_Crop (per-row random crop of audio).

Reference semantics:
    out[i, j] = audio[i, j]             for j <  starts[i]
    out[i, j] = audio[i, j + CROP_NUM]  for j >= starts[i]

Strategy (pure data parallel, 16 rows per core across 8 cores):
out[i] is an elementwise select between the row read at offset 0
(identity) and the row read at offset CROP (shifted), keyed on
global position < starts[i].  Per row: two plain strided DMA loads
into [116, 2048] SBUF tiles (no indirection — the shifted view is just
audio[i, CROP : CROP + 116*2048] reshaped), one tensor_scalar is_lt
building the mask from a precomputed global-position iota against the
row's start (per-partition scalar), one copy_predicated blending the
identity values over the shifted tile, one store.  Handles every lane
including the straddling block exactly — no boundary fixup, no host
splice.  The last row's shifted load is split to stay in bounds; other
rows over-read into the next row (harmless, host-trimmed).

A single ExternalOutput tensor is load-bearing: each extra output costs
~85ms of axon-relay dispatch overhead per call (measured), dwarfing the
~0.3ms device time.  Device HBM traffic/row = read ~2x OUT_LEN + write
OUT_LEN; at ~200GB/s per-core DMA this is ~0.25ms, invisible under the
~41ms dispatch floor.  Inputs are zero-copy views of the caller's audio.
"""

import numpy as np

import concourse.bacc as bacc
import concourse.mybir as mybir
from concourse import bass_utils
from concourse.tile import TileContext

# Problem constants (hardcoded per harness contract).
B = 128
L = 262144
CROP = 26214
OUT_LEN = L - CROP  # 235930
N_CORES = 8
R = B // N_CORES  # 16 rows per core

W = 2048                      # block width; L == 128 * W
N_BLK = OUT_LEN // W + 1      # 116 blocks cover one output row
N_FULL = OUT_LEN // W         # 115 full blocks
TAIL = OUT_LEN - N_FULL * W   # 410
PADW = N_BLK * W              # out row padded to 237568 so every row
                              # store is one 8KB-aligned [116, W] DMA

_programs = {}


def _build_program(reps: int = 1):
    """Build the single SPMD Bass/Tile program (shared by all 8 cores).

    reps > 1 wraps the body in an on-device For_i loop for benchmarking
    (isolates device time from the axon dispatch overhead).
    """
    if reps in _programs:
        return _programs[reps]
    nc = bacc.Bacc("TRN2", target_bir_lowering=False, debug=False)

    audio = nc.dram_tensor(
        "audio", [R * L], mybir.dt.float32, kind="ExternalInput"
    ).ap()
    s_rep = nc.dram_tensor(
        "s_rep", [N_BLK, R], mybir.dt.float32, kind="ExternalInput"
    ).ap()
    glob_pos = nc.dram_tensor(
        "glob_pos", [N_BLK, W], mybir.dt.float32, kind="ExternalInput"
    ).ap()
    out = nc.dram_tensor(
        "out", [R, PADW], mybir.dt.float32, kind="ExternalOutput"
    ).ap()

    with TileContext(nc) as tc:
        with (
            tc.tile_pool(name="consts", bufs=1) as consts,
            tc.tile_pool(name="work", bufs=4) as work,
        ):
            s_rep_sb = consts.tile([N_BLK, R], mybir.dt.float32)
            glob_pos_sb = consts.tile([N_BLK, W], mybir.dt.float32)
            nc.sync.dma_start(out=s_rep_sb[:], in_=s_rep[:])
            nc.sync.dma_start(out=glob_pos_sb[:], in_=glob_pos[:])

            def body():
                for i in range(R):
                    t_sh = work.tile([N_BLK, W], mybir.dt.float32, tag="sh")
                    t_id = work.tile([N_BLK, W], mybir.dt.float32, tag="id")
                    mask = work.tile([N_BLK, W], mybir.dt.uint8, tag="mask")
                    # shifted view: audio[i*L + CROP + k*W + j]
                    base = i * L + CROP
                    if i < R - 1:
                        # over-reads 1638 elems into row i+1 (lane 115,
                        # cols >= 410): harmless, host-trimmed
                        src = audio[base : base + N_BLK * W].rearrange(
                            "(p w) -> p w", w=W
                        )
                        nc.sync.dma_start(out=t_sh[:], in_=src)
                    else:
                        # last row: stay inside the input tensor
                        src = audio[base : base + N_FULL * W].rearrange(
                            "(p w) -> p w", w=W
                        )
                        nc.sync.dma_start(out=t_sh[:N_FULL, :], in_=src)
                        tail = audio[
                            base + N_FULL * W : base + N_FULL * W + TAIL
                        ].rearrange("(p w) -> p w", w=TAIL)
                        nc.sync.dma_start(
                            out=t_sh[N_FULL : N_FULL + 1, :TAIL], in_=tail
                        )
                        # lane 115 cols >= 410 are never selected by a
                        # valid mask, but keep them defined
                        nc.sync.dma_start(
                            out=t_sh[N_FULL : N_FULL + 1, TAIL:],
                            in_=audio[i * L : i * L + W - TAIL].rearrange(
                                "(p w) -> p w", w=W - TAIL
                            ),
                        )
                    # identity view: audio[i*L + k*W + j]
                    src_id = audio[i * L : i * L + N_BLK * W].rearrange(
                        "(p w) -> p w", w=W
                    )
                    nc.sync.dma_start(out=t_id[:], in_=src_id)
                    # mask = (k*W + j) < starts[i]  -> take identity there
                    nc.vector.tensor_scalar(
                        mask[:], glob_pos_sb[:], s_rep_sb[:, i : i + 1],
                        None, mybir.AluOpType.is_lt,
                    )
                    nc.vector.copy_predicated(t_sh[:], mask[:], t_id[:])
                    dst = out[i, :].rearrange("(p w) -> p w", w=W)
                    nc.sync.dma_start(out=dst, in_=t_sh[:])

            if reps == 1:
                body()
            else:
                with tc.For_i(0, reps, 1):
                    body()

    nc.compile()
    _programs[reps] = nc
    return nc


_GLOB_POS = None


def _host_inputs(audio: np.ndarray, starts: np.ndarray):
    """Shard per core: audio slices are zero-copy views; consts are tiny."""
    global _GLOB_POS
    audio = np.ascontiguousarray(audio, dtype=np.float32)
    starts = np.asarray(starts, dtype=np.int32)

    if _GLOB_POS is None:
        _GLOB_POS = (
            np.arange(N_BLK, dtype=np.float32)[:, None] * W
            + np.arange(W, dtype=np.float32)[None, :]
        )  # [116, 2048], exact in f32 (max 237567 < 2^24)

    in_maps = []
    for c in range(N_CORES):
        rows = slice(c * R, (c + 1) * R)
        s_rep = np.broadcast_to(
            starts[rows].astype(np.float32)[None, :], (N_BLK, R)
        ).copy()
        in_maps.append(
            {
                "audio": audio[rows].reshape(-1),  # zero-copy view
                "s_rep": s_rep,
                "glob_pos": _GLOB_POS,
            }
        )
    return in_maps


def _unshard(results):
    out = np.empty((B, OUT_LEN), dtype=np.float32)
    for c in range(N_CORES):
        out[c * R : (c + 1) * R] = results[c]["out"][:, :OUT_LEN]
    return out


def kernel(audio: np.ndarray, starts: np.ndarray) -> np.ndarray:
    nc = _build_program()
    in_maps = _host_inputs(audio, starts)
    res = bass_utils.run_bass_kernel_spmd(
        nc, in_maps, core_ids=list(range(N_CORES))
    )
    kernel.last_results = res
    return _unshard(res.results)
